# revision 5
# baseline (speedup 1.0000x reference)
"""AdaptiveFeaturePooling (cumulative-rescale ROI-align pyramid max-pool) on
8 TRN2 NeuronCores.

Reference semantics (see problem): for i in 3..0 the ROI box tensor is
*cumulatively* rescaled by 2**i * 28 and roi_align'd (14x14 bins, sampling
ratio 2, torchvision aligned=False) against pyramid level i; results are
max-combined.  The cumulative rescale makes nearly every sample point land
out of bounds (contributing exact zeros), so per ROI and level the pooled
map is a sparse bilinear combination of feature pixels that can be written
as Ay[r] @ F[c] @ Ax[r].T with per-ROI axis matrices [14, L] (the 2x2 bin
average folded in).  We fold both axes into one dense operand
B[r][(h,w), (py,px)] = Ay[py,h] * Ax[px,w] and compute, per active ROI,
out[c, q] = sum_hw F[c, hw] * B[hw, q] as K-tiled PE matmuls (K=112,
M=128 channels, N=196) with fp32 PSUM accumulation, then ReLU (the
max with the all-zero levels) on the PSUM drain.

Sharding: ROIs are permuted so that every core owns 64 output slots with
its (at most J) compute-active ROIs in the leading slots; inactive slots
are zero-filled by large SBUF->HBM DMAs.  Feature level 3 is replicated
(0.8MB); levels 0-2 are untouched by the device unless a (freak) input
makes them active, in which case those contributions are merged on host.
"""

import numpy as np

ROI_SIZE = 14
BASE_SIZE = 28
SR = 2
N_CORES = 8
R_TOTAL = 512
C = 256
Q = ROI_SIZE * ROI_SIZE  # 196
LEVEL_HW = {0: 224, 1: 112, 2: 56, 3: 28}
KT = 112  # K-tile (partition) size for the hw contraction


# ----------------------------------------------------------------------------
# host-side exact float32 reimplementation of the coordinate math
# ----------------------------------------------------------------------------

def _prep_coord_np(c, L):
    """float32-exact port of reference._prep_coord."""
    c = c.astype(np.float32, copy=False)
    valid = (c >= np.float32(-1.0)) & (c <= np.float32(L))
    c = np.clip(c, np.float32(0.0), np.float32(L - 1))
    lo = np.floor(c)
    frac = (c - lo).astype(np.float32)
    lo_i = lo.astype(np.int32)
    hi_i = lo_i + 1
    at_edge = lo_i >= L - 1
    lo_i = np.where(at_edge, L - 1, lo_i)
    hi_i = np.where(at_edge, L - 1, hi_i)
    frac = np.where(at_edge, np.float32(0.0), frac)
    return lo_i, hi_i, frac, valid


def _axis_mats(c1, c2, L):
    """Per-ROI interpolation matrix A [R, 14, L] for one axis, float32 math
    identical to the reference, with the 2x2 bin average folded in."""
    R = c1.shape[0]
    G = ROI_SIZE * SR
    steps = ((np.arange(G, dtype=np.float32) + np.float32(0.5)) /
             np.float32(SR)).astype(np.float32)
    roi_l = np.maximum(c2 - c1, np.float32(1.0)).astype(np.float32)
    scale = (roi_l / np.float32(ROI_SIZE)).astype(np.float32)
    cs = (c1[:, None] + steps[None, :] * scale[:, None]).astype(np.float32)
    lo_i, hi_i, frac, valid = _prep_coord_np(cs, L)
    A = np.zeros((R, G, L), dtype=np.float32)
    rr = np.arange(R)[:, None]
    gg = np.arange(G)[None, :]
    v = valid.astype(np.float32)
    np.add.at(A, (rr, gg, lo_i), (np.float32(1.0) - frac) * v)
    np.add.at(A, (rr, gg, hi_i), frac * v)
    A = np.float32(0.5) * (A[:, 0::SR, :] + A[:, 1::SR, :])
    return A


def _make_mats(rois):
    """level -> (Ay [R,14,H], Ax [R,14,W]) with the cumulative rescale."""
    mats = {}
    r = rois.astype(np.float32, copy=True)
    for i in range(3, -1, -1):
        r = (r * np.float32(2.0 ** i * BASE_SIZE)).astype(np.float32)
        L = LEVEL_HW[i]
        Ax = _axis_mats(r[:, 0], r[:, 2], L)
        Ay = _axis_mats(r[:, 1], r[:, 3], L)
        mats[i] = (Ay, Ax)
    return mats


def _host_pool_level(feat, Ay, Ax):
    """roi_align for one level/ROI subset on host: [n,14,L]x[C,H,W] -> [n,C,14,14]."""
    f = feat[0]
    return np.einsum('rph,chw,rqw->rcpq', Ay, f, Ax, optimize=True)


# ----------------------------------------------------------------------------
# device program
# ----------------------------------------------------------------------------

_PROGRAM_CACHE = {}


def _build_program(J, n_k):
    """One SPMD Bass program: J compute jobs (level-3 ROI-align matmuls)
    in slots 0..J-1, zero-fill for slots J..63."""
    import concourse.bass as bass
    import concourse.mybir as mybir
    from concourse.tile import TileContext

    f32 = mybir.dt.float32
    nc = bass.Bass()
    fpack = nc.declare_dram_parameter("fpack", [KT, n_k * C], f32, isOutput=False)
    if J:
        bpack = nc.declare_dram_parameter("bpack", [J, KT, n_k * Q], f32,
                                          isOutput=False)
    out = nc.declare_dram_parameter("out", [64, C, Q], f32, isOutput=True)
    out_flat = out.reshape([64 * C * Q])

    ZCOLS = 3136  # 8 output slots worth of zeros: [128, 3136] = 1.6MB
    with TileContext(nc) as tc:
        with tc.tile_pool(name="pool", bufs=1) as cpool, \
             tc.tile_pool(name="bt", bufs=2) as bpool, \
             tc.tile_pool(name="st", bufs=4) as spool, \
             tc.tile_pool(name="psum", bufs=4, space="PSUM") as ppool:
            ft = cpool.tile([KT, n_k * C], f32)
            nc.sync.dma_start(ft[:], fpack[:])
            bts = []
            for j in range(J):
                bt = bpool.tile([KT, n_k * Q], f32, tag="bt")
                nc.sync.dma_start(bt[:], bpack[j])
                bts.append(bt)

            # zero-fill slots J..63 (large streaming stores from one
            # zeroed SBUF tile)
            zt = cpool.tile([128, ZCOLS], f32)
            nc.vector.memset(zt[:], 0.0)
            off = J * C * Q
            total = 64 * C * Q
            chunk = 128 * ZCOLS
            while off < total:
                n = min(chunk, total - off)
                rows = n // ZCOLS
                if rows * ZCOLS == n and rows > 0:
                    src = zt[0:rows, :]
                else:
                    src = zt[0:1, 0:n]  # small ragged tail (not hit for our sizes)
                nc.sync.dma_start(out_flat[off:off + n], src)
                off += n

            for j in range(J):
                bt = bts[j]
                for cb in range(2):
                    ps = ppool.tile([128, Q], f32, tag="ps")
                    for k in range(n_k):
                        nc.tensor.matmul(
                            ps[:, :],
                            ft[:, k * C + cb * 128: k * C + cb * 128 + 128],
                            bt[:, k * Q: (k + 1) * Q],
                            start=(k == 0),
                            stop=(k == n_k - 1),
                        )
                    st = spool.tile([128, Q], f32, tag="st")
                    nc.scalar.activation(st[:], ps[:],
                                         mybir.ActivationFunctionType.Relu)
                    # computed-slot stores ride the ACT HWDGE queue so they
                    # don't queue behind the zero-fill stream on SP
                    nc.scalar.dma_start(out[j, cb * 128:(cb + 1) * 128, :], st[:])
    _legalize_single_wait(nc, mybir)
    return nc


def _legalize_single_wait(nc, mybir):
    """This walrus build encodes at most ONE semaphore wait per instruction;
    Tile's sem assignment attaches several.  Spill extras onto dedicated
    same-engine nops placed immediately before the instruction (engines
    execute their instructions in block order, so the waits still all
    complete before the original instruction issues)."""
    uid = 0
    for f in nc.m.functions:
        for bb in f.blocks:
            il = bb.instructions
            new = []
            changed = False
            for ins in il:
                si = ins.sync_info
                if si is not None and si.on_wait and len(si.on_wait) > 1:
                    waits = list(si.on_wait)
                    for w in waits[:-1]:
                        nop = mybir.InstNoOp(
                            name=f"splitwait-{uid}",
                            engine=ins.engine,
                            sync_info=mybir.SyncInfo(on_wait=[w], on_update=[]),
                            bass_nofuse=True,
                        )
                        uid += 1
                        try:
                            nc.register_instruction(nop, overwrite=True)
                        except Exception:
                            pass
                        new.append(nop)
                    ins.sync_info = mybir.SyncInfo(
                        on_wait=[waits[-1]], on_update=list(si.on_update))
                    changed = True
                new.append(ins)
            if changed:
                bb.instructions = new


# ----------------------------------------------------------------------------
# entry point
# ----------------------------------------------------------------------------

def _plan(rois):
    """Compute per-level activity and the core/slot assignment."""
    mats = _make_mats(rois)
    active = {}
    for lvl in range(4):
        Ay, Ax = mats[lvl]
        nz = (np.abs(Ay).sum(axis=(1, 2)) > 0) & (np.abs(Ax).sum(axis=(1, 2)) > 0)
        active[lvl] = nz
    d_rois = np.where(active[3])[0]          # device-computed (level 3)
    host_lvls = {lvl: np.where(active[lvl])[0] for lvl in (0, 1, 2)}
    all4 = active[0] & active[1] & active[2] & active[3]
    return mats, active, d_rois, host_lvls, np.where(all4)[0]


def _run_device(feat3, rois, mats, d_rois, trace=False):
    """Returns (full_out [512, C, Q] float32, exec_info)."""
    from concourse.bass_utils import run_bass_kernel_spmd

    Ay3, Ax3 = mats[3]
    J = int(np.ceil(len(d_rois) / N_CORES)) if len(d_rois) else 0
    n_k = (LEVEL_HW[3] * LEVEL_HW[3]) // KT  # 7

    # per-core job lists (round-robin over active ROIs), padded with
    # inactive ROIs (zero B -> zero output, which is their true value)
    jobs = [list(map(int, d_rois[i::N_CORES])) for i in range(N_CORES)]
    used = set(map(int, d_rois))
    spare = [r for r in range(R_TOTAL) if r not in used]
    si = 0
    slots = []
    for i in range(N_CORES):
        pad = J - len(jobs[i])
        take, si = spare[si:si + pad], si + pad
        jobs[i] = jobs[i] + take
    rest = [r for r in spare[si:]]
    ri = 0
    for i in range(N_CORES):
        fill = 64 - J
        slots.append(jobs[i] + rest[ri:ri + fill])
        ri += fill
    assert ri == len(rest)
    perm = np.array([r for s in slots for r in s], dtype=np.int64)
    assert len(np.unique(perm)) == R_TOTAL

    # fpack: [112, 7*256], fpack[p, k*C+c] = feat3[0, c, k*112+p]
    f3 = np.ascontiguousarray(feat3[0].astype(np.float32, copy=False))
    f3hw_c = f3.reshape(C, -1).T                      # [784, 256]
    fpack = np.ascontiguousarray(
        f3hw_c.reshape(n_k, KT, C).transpose(1, 0, 2).reshape(KT, n_k * C))

    in_maps = []
    for i in range(N_CORES):
        m = {"fpack": fpack}
        if J:
            bp = np.zeros((J, KT, n_k * Q), dtype=np.float32)
            for j, r in enumerate(jobs[i]):
                if r in used:
                    B = np.einsum('ph,qw->hwpq', Ay3[r], Ax3[r]
                                  ).reshape(n_k, KT, Q)
                    bp[j] = B.transpose(1, 0, 2).reshape(KT, n_k * Q)
            m["bpack"] = bp
        in_maps.append(m)

    key = (J, n_k)
    if key not in _PROGRAM_CACHE:
        _PROGRAM_CACHE[key] = _build_program(J, n_k)
    nc = _PROGRAM_CACHE[key]

    res = run_bass_kernel_spmd(nc, in_maps, core_ids=list(range(N_CORES)),
                               trace=trace)
    full = np.empty((R_TOTAL, C, Q), dtype=np.float32)
    for i in range(N_CORES):
        full[np.asarray(slots[i], dtype=np.int64)] = res.results[i]["out"]
    return full, res


def kernel(feat0, feat1, feat2, feat3, rois, _trace=False, _return_info=False):
    import os
    feats = {0: feat0, 1: feat1, 2: feat2, 3: feat3}
    rois = np.ascontiguousarray(np.asarray(rois, dtype=np.float32))
    try:
        mats, active, d_rois, host_lvls, all4 = _plan(rois)
        full, info = _run_device(np.asarray(feat3, dtype=np.float32), rois,
                                 mats, d_rois, trace=_trace)

        # merge (host) contributions from levels 0-2 -- empty for the real
        # input distribution, but keeps the kernel correct in general
        for lvl in (2, 1, 0):
            idx = host_lvls[lvl]
            if len(idx):
                Ay, Ax = mats[lvl]
                p = _host_pool_level(np.asarray(feats[lvl], dtype=np.float32),
                                     Ay[idx], Ax[idx]).reshape(len(idx), C, Q)
                full[idx] = np.maximum(full[idx], p)
        # a ROI active at all four levels must not get the implicit relu
        if len(all4):
            pooled = None
            for lvl in (3, 2, 1, 0):
                Ay, Ax = mats[lvl]
                p = _host_pool_level(np.asarray(feats[lvl], dtype=np.float32),
                                     Ay[all4], Ax[all4]).reshape(len(all4), C, Q)
                pooled = p if pooled is None else np.maximum(pooled, p)
            full[all4] = pooled
        out = full.reshape(R_TOTAL, C, ROI_SIZE, ROI_SIZE)
        if _return_info:
            return out, info
        return out
    except Exception:
        if os.environ.get("KERNEL_NO_FALLBACK"):
            raise
        # pure-host fallback (slow but correct)
        out = _host_reference(feat0, feat1, feat2, feat3, rois)
        if _return_info:
            return out, None
        return out


def _host_reference(feat0, feat1, feat2, feat3, rois):
    mats = _make_mats(np.asarray(rois, dtype=np.float32))
    feats = {0: feat0, 1: feat1, 2: feat2, 3: feat3}
    full = None
    for lvl in (3, 2, 1, 0):
        Ay, Ax = mats[lvl]
        nz = np.where((np.abs(Ay).sum(axis=(1, 2)) > 0)
                      & (np.abs(Ax).sum(axis=(1, 2)) > 0))[0]
        p = np.zeros((R_TOTAL, C, Q), dtype=np.float32)
        if len(nz):
            p[nz] = _host_pool_level(np.asarray(feats[lvl], dtype=np.float32),
                                     Ay[nz], Ax[nz]).reshape(len(nz), C, Q)
        full = p if full is None else np.maximum(full, p)
    return full.reshape(R_TOTAL, C, ROI_SIZE, ROI_SIZE)


# revision 32
# speedup vs baseline: 1.2047x; 1.2047x over previous
"""AdaptiveFeaturePooling (cumulative-rescale ROI-align pyramid max-pool) on
8 TRN2 NeuronCores.

Reference semantics (see problem): for i in 3..0 the ROI box tensor is
*cumulatively* rescaled by 2**i * 28 and roi_align'd (14x14 bins, sampling
ratio 2, torchvision aligned=False) against pyramid level i; results are
max-combined.  The cumulative rescale makes nearly every sample point land
out of bounds (contributing exact zeros), so per ROI and level the pooled
map is a sparse bilinear combination of feature pixels that can be written
as Ay[r] @ F[c] @ Ax[r].T with per-ROI axis matrices [14, L] (the 2x2 bin
average folded in).  We fold both axes into one dense operand
B[r][(h,w), (py,px)] = Ay[py,h] * Ax[px,w] and compute, per active ROI,
out[c, q] = sum_hw F[c, hw] * B[hw, q] as K-tiled PE matmuls (K=112,
M=128 channels, N=196) with fp32 PSUM accumulation, then ReLU (the
max with the all-zero levels) on the PSUM drain.

Sharding: ROIs are permuted so that every core owns 64 output slots with
its (at most J) compute-active ROIs in the leading slots; inactive slots
are zero-filled by large SBUF->HBM DMAs.  Feature level 3 is replicated
(0.8MB); levels 0-2 are untouched by the device unless a (freak) input
makes them active, in which case those contributions are merged on host.
"""

import numpy as np

ROI_SIZE = 14
BASE_SIZE = 28
SR = 2
N_CORES = 8
R_TOTAL = 512
C = 256
Q = ROI_SIZE * ROI_SIZE  # 196
LEVEL_HW = {0: 224, 1: 112, 2: 56, 3: 28}
KT = 112  # K-tile (partition) size for the hw contraction


# ----------------------------------------------------------------------------
# host-side exact float32 reimplementation of the coordinate math
# ----------------------------------------------------------------------------

def _prep_coord_np(c, L):
    """float32-exact port of reference._prep_coord."""
    c = c.astype(np.float32, copy=False)
    valid = (c >= np.float32(-1.0)) & (c <= np.float32(L))
    c = np.clip(c, np.float32(0.0), np.float32(L - 1))
    lo = np.floor(c)
    frac = (c - lo).astype(np.float32)
    lo_i = lo.astype(np.int32)
    hi_i = lo_i + 1
    at_edge = lo_i >= L - 1
    lo_i = np.where(at_edge, L - 1, lo_i)
    hi_i = np.where(at_edge, L - 1, hi_i)
    frac = np.where(at_edge, np.float32(0.0), frac)
    return lo_i, hi_i, frac, valid


def _axis_mats(c1, c2, L):
    """Per-ROI interpolation matrix A [R, 14, L] for one axis, float32 math
    identical to the reference, with the 2x2 bin average folded in."""
    R = c1.shape[0]
    G = ROI_SIZE * SR
    steps = ((np.arange(G, dtype=np.float32) + np.float32(0.5)) /
             np.float32(SR)).astype(np.float32)
    roi_l = np.maximum(c2 - c1, np.float32(1.0)).astype(np.float32)
    scale = (roi_l / np.float32(ROI_SIZE)).astype(np.float32)
    cs = (c1[:, None] + steps[None, :] * scale[:, None]).astype(np.float32)
    lo_i, hi_i, frac, valid = _prep_coord_np(cs, L)
    A = np.zeros((R, G, L), dtype=np.float32)
    rr = np.arange(R)[:, None]
    gg = np.arange(G)[None, :]
    v = valid.astype(np.float32)
    np.add.at(A, (rr, gg, lo_i), (np.float32(1.0) - frac) * v)
    np.add.at(A, (rr, gg, hi_i), frac * v)
    A = np.float32(0.5) * (A[:, 0::SR, :] + A[:, 1::SR, :])
    return A


def _make_mats(rois):
    """level -> (Ay [R,14,H], Ax [R,14,W]) with the cumulative rescale."""
    mats = {}
    r = rois.astype(np.float32, copy=True)
    for i in range(3, -1, -1):
        r = (r * np.float32(2.0 ** i * BASE_SIZE)).astype(np.float32)
        L = LEVEL_HW[i]
        Ax = _axis_mats(r[:, 0], r[:, 2], L)
        Ay = _axis_mats(r[:, 1], r[:, 3], L)
        mats[i] = (Ay, Ax)
    return mats


def _host_pool_level(feat, Ay, Ax):
    """roi_align for one level/ROI subset on host: [n,14,L]x[C,H,W] -> [n,C,14,14]."""
    f = feat[0]
    return np.einsum('rph,chw,rqw->rcpq', Ay, f, Ax, optimize=True)


# ----------------------------------------------------------------------------
# device program
# ----------------------------------------------------------------------------

_PROGRAM_CACHE = {}

# 16-bit inputs halve the fpack/bpack HBM reads and enable fast weight load;
# PSUM accumulation stays fp32.  fp16 (10 mantissa bits) keeps rel err ~3e-4;
# the feature values (randn, |x| < 6) and weights (<= 1) are far from fp16
# range limits.
COMPUTE_F16 = True


def _build_program(J, n_k):
    """One SPMD Bass program: J compute jobs (level-3 ROI-align matmuls)
    in slots 0..J-1, zero-fill for slots J..63.

    Layout choices (from trace analysis):
      * zero-fill rides the SP HWDGE ring alone; input loads + computed
        stores ride the ACT ring, so stores don't FIFO behind 12MB of
        zeros.
      * per (cb, k): one LDWEIGHTS shared by all jobs; jobs are batched
        along the moving free dim (jobs x 196 columns, split at <=392
        to stay inside one PSUM bank) -> fewer, larger matmuls.
      * ReLU on DVE (no ACT tables to load).
    """
    import concourse.bass as bass
    import concourse.mybir as mybir
    from concourse.tile import TileContext

    f32 = mybir.dt.float32
    cdt = mybir.dt.float16 if COMPUTE_F16 else f32
    nc = bass.Bass()
    fpack = nc.declare_dram_parameter("fpack", [KT, n_k * C], cdt, isOutput=False)
    if J:
        # bpack[p, (k, j, q)] = B_j[k*KT+p, q]
        bpack = nc.declare_dram_parameter("bpack", [KT, n_k * J * Q], cdt,
                                          isOutput=False)
    out = nc.declare_dram_parameter("out", [64, C, Q], f32, isOutput=True)
    out_flat = out.reshape([64 * C * Q])

    ZCOLS = 1960  # 5 output slots worth of zeros: [128, 1960] = 1MB
    # job batches along the moving dim: [0,2) -> N=392, [2,3) -> N=196, ...
    jb = []
    j0 = 0
    while j0 < J:
        j1 = min(j0 + 2, J)
        jb.append((j0, j1))
        j0 = j1

    with TileContext(nc) as tc:
        with tc.tile_pool(name="pool", bufs=1) as cpool, \
             tc.tile_pool(name="st", bufs=8) as spool, \
             tc.tile_pool(name="psum", bufs=1, space="PSUM") as ppool:
            # zero tile first so the big store stream starts ASAP
            zt = cpool.tile([128, ZCOLS], f32)
            nc.vector.memset(zt[:, 0:ZCOLS // 2], 0.0)
            nc.gpsimd.memset(zt[:, ZCOLS // 2:], 0.0)

            ft = cpool.tile([KT, n_k * C], cdt)
            nc.scalar.dma_start(ft[:], fpack[:])
            if J:
                bt = cpool.tile([KT, n_k * J * Q], cdt)
                nc.scalar.dma_start(bt[:], bpack[:])

            # zero-fill slots J..63 on the SP ring
            off = J * C * Q
            total = 64 * C * Q
            chunk = 128 * ZCOLS
            while off < total:
                n = min(chunk, total - off)
                assert n % 128 == 0 and n // 128 <= ZCOLS
                src = zt[0:128, 0:n // 128]
                nc.sync.dma_start(out_flat[off:off + n], src)
                off += n

            # PE: psum[j][cb] accumulates over k; lhsT shared across jobs
            pss = {}
            for (a, b) in jb:
                for cb in range(2):
                    pss[(a, cb)] = ppool.tile([128, (b - a) * Q], f32,
                                              name=f"ps{a}_{cb}",
                                              tag=f"ps{a}_{cb}")
            for cb in range(2):
                for k in range(n_k):
                    lhsT = ft[:, k * C + cb * 128: k * C + cb * 128 + 128]
                    for (a, b) in jb:
                        nc.tensor.matmul(
                            pss[(a, cb)][:, :],
                            lhsT,
                            bt[:, (k * J + a) * Q: (k * J + b) * Q],
                            start=(k == 0),
                            stop=(k == n_k - 1),
                        )
            for (a, b) in jb:
                for cb in range(2):
                    for j in range(a, b):
                        st = spool.tile([128, Q], f32, tag="st")
                        nc.vector.tensor_relu(
                            st[:], pss[(a, cb)][:, (j - a) * Q:(j - a + 1) * Q])
                        nc.scalar.dma_start(
                            out[j, cb * 128:(cb + 1) * 128, :], st[:])
    _legalize_single_wait(nc, mybir)
    return nc


def _build_program_block(J, n_k, zc=1960, use_gpsimd=False,
                         strip_start_barrier=True):
    """Raw Block-mode version (manual semaphores) — skips TileContext's
    ~11.6us preamble/EVSEM-barrier overhead.  Same dataflow as
    _build_program; see its docstring.

    strip_start_barrier removes Bass.__init__'s const-AP memsets (on the
    slow-booting GpSimd Q7) and the all-engine start barrier; nothing in
    this program reads the const APs, and all cross-engine ordering is by
    ascending semaphores, so engines may start as soon as they boot.
    Semaphores are re-zeroed after the exit barrier so re-executing the
    loaded NEFF stays correct."""
    import concourse.bass as bass
    import concourse.mybir as mybir

    f32 = mybir.dt.float32
    cdt = mybir.dt.float16 if COMPUTE_F16 else f32
    nc = bass.Bass()
    fpack = nc.declare_dram_parameter("fpack", [KT, n_k * C], cdt, isOutput=False)
    if J:
        bpack = nc.declare_dram_parameter("bpack", [KT, n_k * J * Q], cdt,
                                          isOutput=False)
    out = nc.declare_dram_parameter("out", [64, C, Q], f32, isOutput=True)
    out_flat = out.reshape([64 * C * Q])

    ZC = zc
    zt = nc.alloc_sbuf_tensor("zt", [128, ZC], f32)
    ft = nc.alloc_sbuf_tensor("ft", [KT, n_k * C], cdt)
    if J:
        bt = nc.alloc_sbuf_tensor("bt", [KT, n_k * J * Q], cdt)

    # job batches along the moving dim
    jb = []
    j0 = 0
    while j0 < J:
        jb.append((j0, min(j0 + 2, J)))
        j0 = jb[-1][1]
    ps = {}
    for (a, b) in jb:
        for cb in range(2):
            ps[(a, cb)] = nc.alloc_psum_tensor(f"ps{a}_{cb}",
                                               [128, (b - a) * Q], f32)
    # relu order must match the mm_sem increment order: groups complete
    # in (cb, a) order since the k-loop is inside cb
    groups = [(a, b, cb) for cb in range(2) for (a, b) in jb]
    sts = [nc.alloc_sbuf_tensor(f"st{i}", [128, Q], f32)
           for i in range(2 * J)]

    # zero-fill chunks: two small leaders (gated on the partial memset),
    # then big chunks split across the SP and ACT HWDGE rings
    MS0_ = 490
    zoff_small = []
    zoff_big = []
    off = J * C * Q
    total = 64 * C * Q
    for _ in range(2):
        n = 128 * MS0_
        if off + n <= total:
            zoff_small.append((off, n))
            off += n
    chunk = 128 * ZC
    while off < total:
        n = min(chunk, total - off)
        assert n % 128 == 0 and n // 128 <= ZC
        zoff_big.append((off, n))
        off += n
    act_zeros = zoff_big[:len(zoff_big) // 3] if J else []
    sp_zeros = zoff_big[len(zoff_big) // 3:] if J else zoff_big
    n_zero_dmas = len(zoff_small) + len(act_zeros) + len(sp_zeros)

    with nc.Block() as block, \
         nc.semaphore("ms") as ms, nc.semaphore("ld") as ld, \
         nc.semaphore("mm") as mm, nc.semaphore("rl") as rl, \
         nc.semaphore("zs") as zs, nc.semaphore("ss") as ss:

        # progressive memset: a small leading slice unblocks the first
        # zero-store chunks ~1.2us earlier than waiting for the full tile
        MS0 = 490

        @block.vector
        def _(v):
            v.memset(zt[:, :MS0], 0.0).then_inc(ms, 1)
            v.memset(zt[:, MS0:], 0.0).then_inc(ms, 1)
            gi = 0
            for g, (a, b, cb) in enumerate(groups):
                v.wait_ge(mm, g + 1)
                for j in range(a, b):
                    v.tensor_relu(
                        sts[gi][:],
                        ps[(a, cb)][:, (j - a) * Q:(j - a + 1) * Q],
                    ).then_inc(rl, 1)
                    gi += 1

        if J:
            @block.scalar
            def _(sc):
                sc.dma_start(ft[:], fpack[:]).then_inc(ld, 16)
                sc.dma_start(bt[:], bpack[:]).then_inc(ld, 16)
                if act_zeros:
                    sc.wait_ge(ms, 2)
                    for (off_, n) in act_zeros:
                        sc.dma_start(out_flat[off_:off_ + n],
                                     zt[0:128, 0:n // 128]).then_inc(zs, 16)
                gi = 0
                for (a, b, cb) in groups:
                    for j in range(a, b):
                        sc.wait_ge(rl, gi + 1)
                        sc.dma_start(out[j, cb * 128:(cb + 1) * 128, :],
                                     sts[gi][:]).then_inc(ss, 16)
                        gi += 1
                sc.wait_ge(ss, gi * 16)

            @block.tensor
            def _(t):
                t.wait_ge(ld, 32)
                for g, (a, b, cb) in enumerate(groups):
                    for k in range(n_k):
                        mi = t.matmul(
                            ps[(a, cb)][:, :],
                            ft[:, k * C + cb * 128: k * C + cb * 128 + 128],
                            bt[:, (k * J + a) * Q: (k * J + b) * Q],
                            start=(k == 0),
                            stop=(k == n_k - 1),
                        )
                    mi.then_inc(mm, 1)
        else:
            @block.scalar
            def _(sc):
                sc.dma_start(ft[0:1, 0:1], fpack[0:1, 0:1]).then_inc(ld, 16)
                sc.wait_ge(ld, 16)

        @block.sync
        def _(s):
            s.wait_ge(ms, 1)
            for (off_, n) in zoff_small:
                s.dma_start(out_flat[off_:off_ + n],
                            zt[0:128, 0:n // 128]).then_inc(zs, 16)
            s.wait_ge(ms, 2)
            for (off_, n) in sp_zeros:
                s.dma_start(out_flat[off_:off_ + n],
                            zt[0:128, 0:n // 128]).then_inc(zs, 16)
            s.wait_ge(zs, n_zero_dmas * 16)

    # after the exit barrier every engine is done: re-zero the kernel sems
    # so a second execution of the same loaded NEFF starts clean
    for sem in (ms, ld, mm, rl, zs, ss):
        nc.sync.sem_clear(sem)

    if strip_start_barrier:
        bb0 = nc.m.functions[0].blocks[0]
        keep = []
        for ins in bb0.instructions:
            nm = type(ins).__name__
            if nm in ("InstDrain", "InstEventSemaphore"):
                continue
            if nm == "InstMemset" and str(ins.engine) == "EngineType.Pool":
                continue
            keep.append(ins)
        bb0.instructions = keep
    return nc


def _legalize_single_wait(nc, mybir):
    """This walrus build encodes at most ONE semaphore wait per instruction;
    Tile's sem assignment attaches several.  Spill extras onto dedicated
    same-engine nops placed immediately before the instruction (engines
    execute their instructions in block order, so the waits still all
    complete before the original instruction issues)."""
    uid = 0
    for f in nc.m.functions:
        for bb in f.blocks:
            il = bb.instructions
            new = []
            changed = False
            for ins in il:
                si = ins.sync_info
                if si is not None and si.on_wait and len(si.on_wait) > 1:
                    waits = list(si.on_wait)
                    for w in waits[:-1]:
                        nop = mybir.InstNoOp(
                            name=f"splitwait-{uid}",
                            engine=ins.engine,
                            sync_info=mybir.SyncInfo(on_wait=[w], on_update=[]),
                            bass_nofuse=True,
                        )
                        uid += 1
                        try:
                            nc.register_instruction(nop, overwrite=True)
                        except Exception:
                            pass
                        new.append(nop)
                    ins.sync_info = mybir.SyncInfo(
                        on_wait=[waits[-1]], on_update=list(si.on_update))
                    changed = True
                new.append(ins)
            if changed:
                bb.instructions = new


# ----------------------------------------------------------------------------
# entry point
# ----------------------------------------------------------------------------

def _plan(rois):
    """Compute per-level activity and the core/slot assignment."""
    mats = _make_mats(rois)
    active = {}
    for lvl in range(4):
        Ay, Ax = mats[lvl]
        nz = (np.abs(Ay).sum(axis=(1, 2)) > 0) & (np.abs(Ax).sum(axis=(1, 2)) > 0)
        active[lvl] = nz
    d_rois = np.where(active[3])[0]          # device-computed (level 3)
    host_lvls = {lvl: np.where(active[lvl])[0] for lvl in (0, 1, 2)}
    all4 = active[0] & active[1] & active[2] & active[3]
    return mats, active, d_rois, host_lvls, np.where(all4)[0]


def _run_device(feat3, rois, mats, d_rois, trace=False):
    """Returns (full_out [512, C, Q] float32, exec_info)."""
    from concourse.bass_utils import run_bass_kernel_spmd

    Ay3, Ax3 = mats[3]
    J = int(np.ceil(len(d_rois) / N_CORES)) if len(d_rois) else 0
    n_k = (LEVEL_HW[3] * LEVEL_HW[3]) // KT  # 7

    # per-core job lists (round-robin over active ROIs), padded with
    # inactive ROIs (zero B -> zero output, which is their true value)
    jobs = [list(map(int, d_rois[i::N_CORES])) for i in range(N_CORES)]
    used = set(map(int, d_rois))
    spare = [r for r in range(R_TOTAL) if r not in used]
    si = 0
    slots = []
    for i in range(N_CORES):
        pad = J - len(jobs[i])
        take, si = spare[si:si + pad], si + pad
        jobs[i] = jobs[i] + take
    rest = [r for r in spare[si:]]
    ri = 0
    for i in range(N_CORES):
        fill = 64 - J
        slots.append(jobs[i] + rest[ri:ri + fill])
        ri += fill
    assert ri == len(rest)
    perm = np.array([r for s in slots for r in s], dtype=np.int64)
    assert len(np.unique(perm)) == R_TOTAL

    cdt = np.float16 if COMPUTE_F16 else np.float32

    # fpack: [112, 7*256], fpack[p, k*C+c] = feat3[0, c, k*112+p]
    f3 = np.ascontiguousarray(feat3[0].astype(np.float32, copy=False))
    f3hw_c = f3.reshape(C, -1).T                      # [784, 256]
    fpack = np.ascontiguousarray(
        f3hw_c.reshape(n_k, KT, C).transpose(1, 0, 2).reshape(KT, n_k * C)
    ).astype(cdt)

    in_maps = []
    for i in range(N_CORES):
        m = {"fpack": fpack}
        if J:
            bp = np.zeros((KT, n_k, J, Q), dtype=np.float32)
            for j, r in enumerate(jobs[i]):
                if r in used:
                    B = np.einsum('ph,qw->hwpq', Ay3[r], Ax3[r]
                                  ).reshape(n_k, KT, Q)
                    bp[:, :, j, :] = B.transpose(1, 0, 2)
            m["bpack"] = np.ascontiguousarray(
                bp.reshape(KT, n_k * J * Q)).astype(cdt)
        in_maps.append(m)

    key = (J, n_k)
    if key not in _PROGRAM_CACHE:
        _PROGRAM_CACHE[key] = _build_program_block(J, n_k)
    nc = _PROGRAM_CACHE[key]

    res = run_bass_kernel_spmd(nc, in_maps, core_ids=list(range(N_CORES)),
                               trace=trace)
    full = np.empty((R_TOTAL, C, Q), dtype=np.float32)
    for i in range(N_CORES):
        full[np.asarray(slots[i], dtype=np.int64)] = res.results[i]["out"]
    return full, res


def kernel(feat0, feat1, feat2, feat3, rois, _trace=False, _return_info=False):
    import os
    feats = {0: feat0, 1: feat1, 2: feat2, 3: feat3}
    rois = np.ascontiguousarray(np.asarray(rois, dtype=np.float32))
    try:
        mats, active, d_rois, host_lvls, all4 = _plan(rois)
        # the device occasionally reports a transient NRT exec error right
        # after another NEFF crashed/was killed on the same cores; retry
        # before giving up on the device path
        last = None
        for attempt in range(3):
            try:
                full, info = _run_device(np.asarray(feat3, dtype=np.float32),
                                         rois, mats, d_rois, trace=_trace)
                break
            except Exception as e:
                last = e
        else:
            raise last

        # merge (host) contributions from levels 0-2 -- empty for the real
        # input distribution, but keeps the kernel correct in general
        for lvl in (2, 1, 0):
            idx = host_lvls[lvl]
            if len(idx):
                Ay, Ax = mats[lvl]
                p = _host_pool_level(np.asarray(feats[lvl], dtype=np.float32),
                                     Ay[idx], Ax[idx]).reshape(len(idx), C, Q)
                full[idx] = np.maximum(full[idx], p)
        # a ROI active at all four levels must not get the implicit relu
        if len(all4):
            pooled = None
            for lvl in (3, 2, 1, 0):
                Ay, Ax = mats[lvl]
                p = _host_pool_level(np.asarray(feats[lvl], dtype=np.float32),
                                     Ay[all4], Ax[all4]).reshape(len(all4), C, Q)
                pooled = p if pooled is None else np.maximum(pooled, p)
            full[all4] = pooled
        out = full.reshape(R_TOTAL, C, ROI_SIZE, ROI_SIZE)
        if _return_info:
            return out, info
        return out
    except Exception:
        if os.environ.get("KERNEL_NO_FALLBACK"):
            raise
        # pure-host fallback (slow but correct)
        out = _host_reference(feat0, feat1, feat2, feat3, rois)
        if _return_info:
            return out, None
        return out


def _host_reference(feat0, feat1, feat2, feat3, rois):
    mats = _make_mats(np.asarray(rois, dtype=np.float32))
    feats = {0: feat0, 1: feat1, 2: feat2, 3: feat3}
    full = None
    for lvl in (3, 2, 1, 0):
        Ay, Ax = mats[lvl]
        nz = np.where((np.abs(Ay).sum(axis=(1, 2)) > 0)
                      & (np.abs(Ax).sum(axis=(1, 2)) > 0))[0]
        p = np.zeros((R_TOTAL, C, Q), dtype=np.float32)
        if len(nz):
            p[nz] = _host_pool_level(np.asarray(feats[lvl], dtype=np.float32),
                                     Ay[nz], Ax[nz]).reshape(len(nz), C, Q)
        full = p if full is None else np.maximum(full, p)
    return full.reshape(R_TOTAL, C, ROI_SIZE, ROI_SIZE)


# revision 36
# speedup vs baseline: 1.4669x; 1.2176x over previous
"""AdaptiveFeaturePooling (cumulative-rescale ROI-align pyramid max-pool) on
8 TRN2 NeuronCores.

Reference semantics (see problem): for i in 3..0 the ROI box tensor is
*cumulatively* rescaled by 2**i * 28 and roi_align'd (14x14 bins, sampling
ratio 2, torchvision aligned=False) against pyramid level i; results are
max-combined.  The cumulative rescale makes nearly every sample point land
out of bounds (contributing exact zeros), so per ROI and level the pooled
map is a sparse bilinear combination of feature pixels that can be written
as Ay[r] @ F[c] @ Ax[r].T with per-ROI axis matrices [14, L] (the 2x2 bin
average folded in).  We fold both axes into one dense operand
B[r][(h,w), (py,px)] = Ay[py,h] * Ax[px,w] and compute, per active ROI,
out[c, q] = sum_hw F[c, hw] * B[hw, q] as K-tiled PE matmuls (K=112,
M=128 channels, N=196) with fp32 PSUM accumulation, then ReLU (the
max with the all-zero levels) on the PSUM drain.

Sharding: ROIs are permuted so that every core owns 64 output slots with
its (at most J) compute-active ROIs in the leading slots; inactive slots
are zero-filled by large SBUF->HBM DMAs.  Feature level 3 is replicated
(0.8MB); levels 0-2 are untouched by the device unless a (freak) input
makes them active, in which case those contributions are merged on host.
"""

import numpy as np

ROI_SIZE = 14
BASE_SIZE = 28
SR = 2
N_CORES = 8
R_TOTAL = 512
C = 256
Q = ROI_SIZE * ROI_SIZE  # 196
LEVEL_HW = {0: 224, 1: 112, 2: 56, 3: 28}
KT = 112  # K-tile (partition) size for the hw contraction


# ----------------------------------------------------------------------------
# host-side exact float32 reimplementation of the coordinate math
# ----------------------------------------------------------------------------

def _prep_coord_np(c, L):
    """float32-exact port of reference._prep_coord."""
    c = c.astype(np.float32, copy=False)
    valid = (c >= np.float32(-1.0)) & (c <= np.float32(L))
    c = np.clip(c, np.float32(0.0), np.float32(L - 1))
    lo = np.floor(c)
    frac = (c - lo).astype(np.float32)
    lo_i = lo.astype(np.int32)
    hi_i = lo_i + 1
    at_edge = lo_i >= L - 1
    lo_i = np.where(at_edge, L - 1, lo_i)
    hi_i = np.where(at_edge, L - 1, hi_i)
    frac = np.where(at_edge, np.float32(0.0), frac)
    return lo_i, hi_i, frac, valid


def _axis_mats(c1, c2, L):
    """Per-ROI interpolation matrix A [R, 14, L] for one axis, float32 math
    identical to the reference, with the 2x2 bin average folded in."""
    R = c1.shape[0]
    G = ROI_SIZE * SR
    steps = ((np.arange(G, dtype=np.float32) + np.float32(0.5)) /
             np.float32(SR)).astype(np.float32)
    roi_l = np.maximum(c2 - c1, np.float32(1.0)).astype(np.float32)
    scale = (roi_l / np.float32(ROI_SIZE)).astype(np.float32)
    cs = (c1[:, None] + steps[None, :] * scale[:, None]).astype(np.float32)
    lo_i, hi_i, frac, valid = _prep_coord_np(cs, L)
    A = np.zeros((R, G, L), dtype=np.float32)
    rr = np.arange(R)[:, None]
    gg = np.arange(G)[None, :]
    v = valid.astype(np.float32)
    np.add.at(A, (rr, gg, lo_i), (np.float32(1.0) - frac) * v)
    np.add.at(A, (rr, gg, hi_i), frac * v)
    A = np.float32(0.5) * (A[:, 0::SR, :] + A[:, 1::SR, :])
    return A


def _make_mats(rois):
    """level -> (Ay [R,14,H], Ax [R,14,W]) with the cumulative rescale."""
    mats = {}
    r = rois.astype(np.float32, copy=True)
    for i in range(3, -1, -1):
        r = (r * np.float32(2.0 ** i * BASE_SIZE)).astype(np.float32)
        L = LEVEL_HW[i]
        Ax = _axis_mats(r[:, 0], r[:, 2], L)
        Ay = _axis_mats(r[:, 1], r[:, 3], L)
        mats[i] = (Ay, Ax)
    return mats


def _host_pool_level(feat, Ay, Ax):
    """roi_align for one level/ROI subset on host: [n,14,L]x[C,H,W] -> [n,C,14,14]."""
    f = feat[0]
    return np.einsum('rph,chw,rqw->rcpq', Ay, f, Ax, optimize=True)


# ----------------------------------------------------------------------------
# device program
# ----------------------------------------------------------------------------

_PROGRAM_CACHE = {}

# 16-bit inputs halve the fpack/bpack HBM reads and enable fast weight load;
# PSUM accumulation stays fp32.  fp16 (10 mantissa bits) keeps rel err ~3e-4;
# the feature values (randn, |x| < 6) and weights (<= 1) are far from fp16
# range limits.
COMPUTE_F16 = True


def _build_program(J, n_k):
    """One SPMD Bass program: J compute jobs (level-3 ROI-align matmuls)
    in slots 0..J-1, zero-fill for slots J..63.

    Layout choices (from trace analysis):
      * zero-fill rides the SP HWDGE ring alone; input loads + computed
        stores ride the ACT ring, so stores don't FIFO behind 12MB of
        zeros.
      * per (cb, k): one LDWEIGHTS shared by all jobs; jobs are batched
        along the moving free dim (jobs x 196 columns, split at <=392
        to stay inside one PSUM bank) -> fewer, larger matmuls.
      * ReLU on DVE (no ACT tables to load).
    """
    import concourse.bass as bass
    import concourse.mybir as mybir
    from concourse.tile import TileContext

    f32 = mybir.dt.float32
    cdt = mybir.dt.float16 if COMPUTE_F16 else f32
    nc = bass.Bass()
    fpack = nc.declare_dram_parameter("fpack", [KT, n_k * C], cdt, isOutput=False)
    if J:
        # bpack[p, (k, j, q)] = B_j[k*KT+p, q]
        bpack = nc.declare_dram_parameter("bpack", [KT, n_k * J * Q], cdt,
                                          isOutput=False)
    out = nc.declare_dram_parameter("out", [64, C, Q], f32, isOutput=True)
    out_flat = out.reshape([64 * C * Q])

    ZCOLS = 1960  # 5 output slots worth of zeros: [128, 1960] = 1MB
    # job batches along the moving dim: [0,2) -> N=392, [2,3) -> N=196, ...
    jb = []
    j0 = 0
    while j0 < J:
        j1 = min(j0 + 2, J)
        jb.append((j0, j1))
        j0 = j1

    with TileContext(nc) as tc:
        with tc.tile_pool(name="pool", bufs=1) as cpool, \
             tc.tile_pool(name="st", bufs=8) as spool, \
             tc.tile_pool(name="psum", bufs=1, space="PSUM") as ppool:
            # zero tile first so the big store stream starts ASAP
            zt = cpool.tile([128, ZCOLS], f32)
            nc.vector.memset(zt[:, 0:ZCOLS // 2], 0.0)
            nc.gpsimd.memset(zt[:, ZCOLS // 2:], 0.0)

            ft = cpool.tile([KT, n_k * C], cdt)
            nc.scalar.dma_start(ft[:], fpack[:])
            if J:
                bt = cpool.tile([KT, n_k * J * Q], cdt)
                nc.scalar.dma_start(bt[:], bpack[:])

            # zero-fill slots J..63 on the SP ring
            off = J * C * Q
            total = 64 * C * Q
            chunk = 128 * ZCOLS
            while off < total:
                n = min(chunk, total - off)
                assert n % 128 == 0 and n // 128 <= ZCOLS
                src = zt[0:128, 0:n // 128]
                nc.sync.dma_start(out_flat[off:off + n], src)
                off += n

            # PE: psum[j][cb] accumulates over k; lhsT shared across jobs
            pss = {}
            for (a, b) in jb:
                for cb in range(2):
                    pss[(a, cb)] = ppool.tile([128, (b - a) * Q], f32,
                                              name=f"ps{a}_{cb}",
                                              tag=f"ps{a}_{cb}")
            for cb in range(2):
                for k in range(n_k):
                    lhsT = ft[:, k * C + cb * 128: k * C + cb * 128 + 128]
                    for (a, b) in jb:
                        nc.tensor.matmul(
                            pss[(a, cb)][:, :],
                            lhsT,
                            bt[:, (k * J + a) * Q: (k * J + b) * Q],
                            start=(k == 0),
                            stop=(k == n_k - 1),
                        )
            for (a, b) in jb:
                for cb in range(2):
                    for j in range(a, b):
                        st = spool.tile([128, Q], f32, tag="st")
                        nc.vector.tensor_relu(
                            st[:], pss[(a, cb)][:, (j - a) * Q:(j - a + 1) * Q])
                        nc.scalar.dma_start(
                            out[j, cb * 128:(cb + 1) * 128, :], st[:])
    _legalize_single_wait(nc, mybir)
    return nc


def _build_program_block(J, n_k, zc=1960, use_gpsimd=False,
                         strip_start_barrier=True):
    """Raw Block-mode version (manual semaphores) — skips TileContext's
    ~11.6us preamble/EVSEM-barrier overhead.  Same dataflow as
    _build_program; see its docstring.

    strip_start_barrier removes Bass.__init__'s const-AP memsets (on the
    slow-booting GpSimd Q7) and the all-engine start barrier; nothing in
    this program reads the const APs, and all cross-engine ordering is by
    ascending semaphores, so engines may start as soon as they boot.
    Semaphores are re-zeroed after the exit barrier so re-executing the
    loaded NEFF stays correct."""
    import concourse.bass as bass
    import concourse.mybir as mybir

    f32 = mybir.dt.float32
    cdt = mybir.dt.float16 if COMPUTE_F16 else f32
    FCOLS = n_k * C
    BCOLS = n_k * J * Q
    nc = bass.Bass()
    # single input tensor: [112, fpack-cols | bpack-cols] -> one load DMA
    inpack = nc.declare_dram_parameter("inpack", [KT, FCOLS + BCOLS], cdt,
                                       isOutput=False)
    out = nc.declare_dram_parameter("out", [64, C, Q], f32, isOutput=True)
    out_flat = out.reshape([64 * C * Q])

    ZC = zc
    zt = nc.alloc_sbuf_tensor("zt", [128, ZC], f32)
    it = nc.alloc_sbuf_tensor("it", [KT, FCOLS + BCOLS], cdt)

    # job batches along the moving dim
    jb = []
    j0 = 0
    while j0 < J:
        jb.append((j0, min(j0 + 2, J)))
        j0 = jb[-1][1]
    ps = {}
    for (a, b) in jb:
        for cb in range(2):
            ps[(a, cb)] = nc.alloc_psum_tensor(f"ps{a}_{cb}",
                                               [128, (b - a) * Q], f32)
    # relu order must match the mm_sem increment order: groups complete
    # in (cb, a) order since the k-loop is inside cb
    groups = [(a, b, cb) for cb in range(2) for (a, b) in jb]
    sts = [nc.alloc_sbuf_tensor(f"st{i}", [128, Q], f32)
           for i in range(2 * J)]

    # zero-fill chunks: two small leaders (gated on the partial memset),
    # then big chunks split across the SP and ACT HWDGE rings
    MS0_ = 490
    zoff_small = []
    zoff_big = []
    off = J * C * Q
    total = 64 * C * Q
    for _ in range(2):
        n = 128 * MS0_
        if off + n <= total:
            zoff_small.append((off, n))
            off += n
    chunk = 128 * ZC
    while off < total:
        n = min(chunk, total - off)
        assert n % 128 == 0 and n // 128 <= ZC
        zoff_big.append((off, n))
        off += n
    act_zeros = zoff_big[:len(zoff_big) // 3] if J else []
    sp_zeros = zoff_big[len(zoff_big) // 3:] if J else zoff_big
    n_zero_dmas = len(zoff_small) + len(act_zeros) + len(sp_zeros)

    with nc.Block() as block, \
         nc.semaphore("ms") as ms, nc.semaphore("ld") as ld, \
         nc.semaphore("mm") as mm, nc.semaphore("rl") as rl, \
         nc.semaphore("zs") as zs, nc.semaphore("ss") as ss:

        # progressive memset: a small leading slice unblocks the first
        # zero-store chunks ~1.2us earlier than waiting for the full tile
        MS0 = 490

        @block.vector
        def _(v):
            v.memset(zt[:, :MS0], 0.0).then_inc(ms, 1)
            v.memset(zt[:, MS0:], 0.0).then_inc(ms, 1)
            gi = 0
            for g, (a, b, cb) in enumerate(groups):
                v.wait_ge(mm, g + 1)
                for j in range(a, b):
                    v.tensor_relu(
                        sts[gi][:],
                        ps[(a, cb)][:, (j - a) * Q:(j - a + 1) * Q],
                    ).then_inc(rl, 1)
                    gi += 1

        if J:
            @block.scalar
            def _(sc):
                sc.dma_start(it[:], inpack[:]).then_inc(ld, 16)
                if act_zeros:
                    sc.wait_ge(ms, 2)
                    for (off_, n) in act_zeros:
                        sc.dma_start(out_flat[off_:off_ + n],
                                     zt[0:128, 0:n // 128]).then_inc(zs, 16)
                gi = 0
                for (a, b, cb) in groups:
                    for j in range(a, b):
                        sc.wait_ge(rl, gi + 1)
                        sc.dma_start(out[j, cb * 128:(cb + 1) * 128, :],
                                     sts[gi][:]).then_inc(ss, 16)
                        gi += 1
                sc.wait_ge(ss, gi * 16)

            @block.tensor
            def _(t):
                t.wait_ge(ld, 16)
                for g, (a, b, cb) in enumerate(groups):
                    for k in range(n_k):
                        mi = t.matmul(
                            ps[(a, cb)][:, :],
                            it[:, k * C + cb * 128: k * C + cb * 128 + 128],
                            it[:, FCOLS + (k * J + a) * Q:
                               FCOLS + (k * J + b) * Q],
                            start=(k == 0),
                            stop=(k == n_k - 1),
                        )
                    mi.then_inc(mm, 1)
        else:
            @block.scalar
            def _(sc):
                sc.dma_start(it[0:1, 0:1], inpack[0:1, 0:1]).then_inc(ld, 16)
                sc.wait_ge(ld, 16)

        @block.sync
        def _(s):
            s.wait_ge(ms, 1)
            for (off_, n) in zoff_small:
                s.dma_start(out_flat[off_:off_ + n],
                            zt[0:128, 0:n // 128]).then_inc(zs, 16)
            s.wait_ge(ms, 2)
            for (off_, n) in sp_zeros:
                s.dma_start(out_flat[off_:off_ + n],
                            zt[0:128, 0:n // 128]).then_inc(zs, 16)
            s.wait_ge(zs, n_zero_dmas * 16)

    # after the exit barrier every engine is done: re-zero the kernel sems
    # so a second execution of the same loaded NEFF starts clean
    for sem in (ms, ld, mm, rl, zs, ss):
        nc.sync.sem_clear(sem)

    if strip_start_barrier:
        bb0 = nc.m.functions[0].blocks[0]
        keep = []
        for ins in bb0.instructions:
            nm = type(ins).__name__
            if nm in ("InstDrain", "InstEventSemaphore"):
                continue
            if nm == "InstMemset" and str(ins.engine) == "EngineType.Pool":
                continue
            keep.append(ins)
        bb0.instructions = keep
    return nc


def _legalize_single_wait(nc, mybir):
    """This walrus build encodes at most ONE semaphore wait per instruction;
    Tile's sem assignment attaches several.  Spill extras onto dedicated
    same-engine nops placed immediately before the instruction (engines
    execute their instructions in block order, so the waits still all
    complete before the original instruction issues)."""
    uid = 0
    for f in nc.m.functions:
        for bb in f.blocks:
            il = bb.instructions
            new = []
            changed = False
            for ins in il:
                si = ins.sync_info
                if si is not None and si.on_wait and len(si.on_wait) > 1:
                    waits = list(si.on_wait)
                    for w in waits[:-1]:
                        nop = mybir.InstNoOp(
                            name=f"splitwait-{uid}",
                            engine=ins.engine,
                            sync_info=mybir.SyncInfo(on_wait=[w], on_update=[]),
                            bass_nofuse=True,
                        )
                        uid += 1
                        try:
                            nc.register_instruction(nop, overwrite=True)
                        except Exception:
                            pass
                        new.append(nop)
                    ins.sync_info = mybir.SyncInfo(
                        on_wait=[waits[-1]], on_update=list(si.on_update))
                    changed = True
                new.append(ins)
            if changed:
                bb.instructions = new


# ----------------------------------------------------------------------------
# entry point
# ----------------------------------------------------------------------------

def _plan(rois):
    """Compute per-level activity and the core/slot assignment."""
    mats = _make_mats(rois)
    active = {}
    for lvl in range(4):
        Ay, Ax = mats[lvl]
        nz = (np.abs(Ay).sum(axis=(1, 2)) > 0) & (np.abs(Ax).sum(axis=(1, 2)) > 0)
        active[lvl] = nz
    d_rois = np.where(active[3])[0]          # device-computed (level 3)
    host_lvls = {lvl: np.where(active[lvl])[0] for lvl in (0, 1, 2)}
    all4 = active[0] & active[1] & active[2] & active[3]
    return mats, active, d_rois, host_lvls, np.where(all4)[0]


def _run_device(feat3, rois, mats, d_rois, trace=False):
    """Returns (full_out [512, C, Q] float32, exec_info)."""
    from concourse.bass_utils import run_bass_kernel_spmd

    Ay3, Ax3 = mats[3]
    J = int(np.ceil(len(d_rois) / N_CORES)) if len(d_rois) else 0
    n_k = (LEVEL_HW[3] * LEVEL_HW[3]) // KT  # 7

    # per-core job lists (round-robin over active ROIs), padded with
    # inactive ROIs (zero B -> zero output, which is their true value)
    jobs = [list(map(int, d_rois[i::N_CORES])) for i in range(N_CORES)]
    used = set(map(int, d_rois))
    spare = [r for r in range(R_TOTAL) if r not in used]
    si = 0
    slots = []
    for i in range(N_CORES):
        pad = J - len(jobs[i])
        take, si = spare[si:si + pad], si + pad
        jobs[i] = jobs[i] + take
    rest = [r for r in spare[si:]]
    ri = 0
    for i in range(N_CORES):
        fill = 64 - J
        slots.append(jobs[i] + rest[ri:ri + fill])
        ri += fill
    assert ri == len(rest)
    perm = np.array([r for s in slots for r in s], dtype=np.int64)
    assert len(np.unique(perm)) == R_TOTAL

    cdt = np.float16 if COMPUTE_F16 else np.float32

    # fpack: [112, 7*256], fpack[p, k*C+c] = feat3[0, c, k*112+p]
    f3 = np.ascontiguousarray(feat3[0].astype(np.float32, copy=False))
    f3hw_c = f3.reshape(C, -1).T                      # [784, 256]
    fpack = np.ascontiguousarray(
        f3hw_c.reshape(n_k, KT, C).transpose(1, 0, 2).reshape(KT, n_k * C)
    ).astype(cdt)

    in_maps = []
    for i in range(N_CORES):
        if J:
            bp = np.zeros((KT, n_k, J, Q), dtype=np.float32)
            for j, r in enumerate(jobs[i]):
                if r in used:
                    B = np.einsum('ph,qw->hwpq', Ay3[r], Ax3[r]
                                  ).reshape(n_k, KT, Q)
                    bp[:, :, j, :] = B.transpose(1, 0, 2)
            inp = np.concatenate(
                [fpack, bp.reshape(KT, n_k * J * Q).astype(cdt)], axis=1)
        else:
            inp = fpack
        in_maps.append({"inpack": np.ascontiguousarray(inp)})

    key = (J, n_k)
    if key not in _PROGRAM_CACHE:
        _PROGRAM_CACHE[key] = _build_program_block(J, n_k)
    nc = _PROGRAM_CACHE[key]

    res = run_bass_kernel_spmd(nc, in_maps, core_ids=list(range(N_CORES)),
                               trace=trace)
    full = np.empty((R_TOTAL, C, Q), dtype=np.float32)
    for i in range(N_CORES):
        full[np.asarray(slots[i], dtype=np.int64)] = res.results[i]["out"]
    return full, res


def kernel(feat0, feat1, feat2, feat3, rois, _trace=False, _return_info=False):
    import os
    feats = {0: feat0, 1: feat1, 2: feat2, 3: feat3}
    rois = np.ascontiguousarray(np.asarray(rois, dtype=np.float32))
    try:
        mats, active, d_rois, host_lvls, all4 = _plan(rois)
        # the device occasionally reports a transient NRT exec error right
        # after another NEFF crashed/was killed on the same cores; retry
        # before giving up on the device path
        last = None
        for attempt in range(3):
            try:
                full, info = _run_device(np.asarray(feat3, dtype=np.float32),
                                         rois, mats, d_rois, trace=_trace)
                break
            except Exception as e:
                last = e
        else:
            raise last

        # merge (host) contributions from levels 0-2 -- empty for the real
        # input distribution, but keeps the kernel correct in general
        for lvl in (2, 1, 0):
            idx = host_lvls[lvl]
            if len(idx):
                Ay, Ax = mats[lvl]
                p = _host_pool_level(np.asarray(feats[lvl], dtype=np.float32),
                                     Ay[idx], Ax[idx]).reshape(len(idx), C, Q)
                full[idx] = np.maximum(full[idx], p)
        # a ROI active at all four levels must not get the implicit relu
        if len(all4):
            pooled = None
            for lvl in (3, 2, 1, 0):
                Ay, Ax = mats[lvl]
                p = _host_pool_level(np.asarray(feats[lvl], dtype=np.float32),
                                     Ay[all4], Ax[all4]).reshape(len(all4), C, Q)
                pooled = p if pooled is None else np.maximum(pooled, p)
            full[all4] = pooled
        out = full.reshape(R_TOTAL, C, ROI_SIZE, ROI_SIZE)
        if _return_info:
            return out, info
        return out
    except Exception:
        if os.environ.get("KERNEL_NO_FALLBACK"):
            raise
        # pure-host fallback (slow but correct)
        out = _host_reference(feat0, feat1, feat2, feat3, rois)
        if _return_info:
            return out, None
        return out


def _host_reference(feat0, feat1, feat2, feat3, rois):
    mats = _make_mats(np.asarray(rois, dtype=np.float32))
    feats = {0: feat0, 1: feat1, 2: feat2, 3: feat3}
    full = None
    for lvl in (3, 2, 1, 0):
        Ay, Ax = mats[lvl]
        nz = np.where((np.abs(Ay).sum(axis=(1, 2)) > 0)
                      & (np.abs(Ax).sum(axis=(1, 2)) > 0))[0]
        p = np.zeros((R_TOTAL, C, Q), dtype=np.float32)
        if len(nz):
            p[nz] = _host_pool_level(np.asarray(feats[lvl], dtype=np.float32),
                                     Ay[nz], Ax[nz]).reshape(len(nz), C, Q)
        full = p if full is None else np.maximum(full, p)
    return full.reshape(R_TOTAL, C, ROI_SIZE, ROI_SIZE)
